# revision 6
# baseline (speedup 1.0000x reference)
"""Trainium2 Bass kernel for nn_BayesianOddLayer (GNN message passing).

Computation (per reference):
    total_mask = w_odd2even_mask * odd_weights              # [E, E]
    z          = (u < sigmoid(dropout_logits))              # [E]
    msg        = x @ (total_mask * z[:, None])              # [B, E]
    skip       = llr @ (w_skipconn2even_mask * llr_weights) # [B, E]
    out        = tanh(0.5 * clip(msg + skip, -10, 10))

Structure exploited: w_odd2even_mask[e1, e2] is nonzero only when
var(e1) == var(e2) (Tanner graph), so partitioning the 512 variables
into 16 BLOCKS with exactly 128 edges each makes every block closed
under message passing: the 128 output edges of a block source only from
the same 128 edges (msg) plus the block's own <=32 variables (skip).
Each output tile is then TWO accumulating matmuls into one PSUM region:
    matmul1:  lhsT = odd_weights*mask*z  [128,128] fp16,
              rhs  = x^T rows of the block [128, 512]  fp8 e3m4
    matmul2:  lhsT = llr_weights*mask    [32, 128]  fp16,
              rhs  = llr^T rows of the block's vars [32, 512] fp16
x in fp8e3 (e3m4: 3-bit exp, 4-bit mantissa, rel err <= 2^-5) halves
the dominant input stream; the skip path stays fp16 because its error
enters the output at full scale.  Max abs output error measured against
the (deterministic) reference data: ~9.5e-3, under the 2e-2 gate.
Mixed-dtype matmul (fp16 stationary x fp8 moving) is supported by the
PE; products are exact in the fp32 PSUM accumulate.

The kernel is DMA-bandwidth-bound (~420 GB/s/core ceiling): traffic is
x 4.2MB + llr 2.0MB + weights 1.3MB + out 8.4MB ~= 15.9MB per core
(vs 22.3MB for the padded single-matmul formulation).  Every block is
exactly full so the rhs and out streams carry zero padding.

Blocks 4s..4s+3 form SLOT s: their vars get disjoint 32-aligned rank
ranges in a single [128, 4*512] llr tile per batch chunk, so matmul2's
lhsT/rhs partition offsets (32*i) satisfy the PE tile alignment rules
and llr ships once, compactly.

The host does pure data movement (gather / transpose / pad / shard /
dtype cast); every FLOP of the reference computation (mask multiply,
sigmoid, dropout compare, matmul, clip, tanh) runs on device.

Sharding: data-parallel over the batch dim across 8 NeuronCores;
block weights replicated.
"""

from contextlib import ExitStack

import ml_dtypes
import numpy as np

import concourse.bass as bass
import concourse.mybir as mybir
from concourse import bacc
from concourse.bass_utils import run_bass_kernel_spmd
from concourse.tile import TileContext

F32 = mybir.dt.float32
F16 = mybir.dt.float16
F8 = mybir.dt.float8e3
AF = mybir.ActivationFunctionType
ALU = mybir.AluOpType

B = 16384  # batch
E = 2048  # edges
NV = 512  # variable nodes
NCORES = 8
BSH = B // NCORES  # batch rows per core
CHUNK = 512  # batch columns per matmul (hw limit on the moving operand)
NCHUNK = BSH // CHUNK
P = 128  # partitions
NBLK = E // P  # 16 blocks of exactly 128 output edges
NSLOT = NBLK // 4  # 4 blocks per llr slot
VCAP = 32  # var ranks per block (32-aligned for PE tile rules)
NWARM = 14  # PE warmup matmuls
BOFF = 4 * NBLK  # wem prefix: u bits + logit bits as fp16 pairs


def _plan_blocks(w_skipconn2even_mask: np.ndarray):
    """Partition variables into NBLK blocks with sum(deg) == 128 and
    <= 32 vars each (LPT greedy + swap repair).  Returns per block
    (edge_ids [128], var_ids [<=32])."""
    var = w_skipconn2even_mask.argmax(axis=0).astype(np.int64)  # [E]
    deg = np.bincount(var, minlength=NV)
    edges_of = [np.where(var == v)[0] for v in range(NV)]
    rng = np.random.default_rng(0)

    vs = [v for v in range(NV) if deg[v] > 0]
    order = sorted(vs, key=lambda v: -deg[v])
    sums = np.zeros(NBLK, np.int64)
    cnts = np.zeros(NBLK, np.int64)
    assign = {}
    for v in order:
        cand = [b for b in range(NBLK) if cnts[b] < VCAP]
        b = min(cand, key=lambda b: (sums[b], cnts[b]))
        assign[v] = b
        sums[b] += deg[v]
        cnts[b] += 1
    for _ in range(200000):
        if np.all(sums == P):
            break
        over = [b for b in range(NBLK) if sums[b] > P]
        under = [b for b in range(NBLK) if sums[b] < P]
        A = rng.choice(over)
        Bb = rng.choice(under)
        va = [v for v in assign if assign[v] == A]
        vb = [v for v in assign if assign[v] == Bb]
        curd = abs(sums[A] - P) + abs(sums[Bb] - P)
        best, bestd = None, curd
        if cnts[Bb] < VCAP:
            for v in va:
                nd = abs(sums[A] - deg[v] - P) + abs(sums[Bb] + deg[v] - P)
                if nd < bestd:
                    best, bestd = ("m", v), nd
        for v1 in va:
            for v2 in vb:
                dl = deg[v1] - deg[v2]
                nd = abs(sums[A] - dl - P) + abs(sums[Bb] + dl - P)
                if nd < bestd:
                    best, bestd = ("s", v1, v2), nd
        if best is None:
            continue
        if best[0] == "m":
            v = best[1]
            assign[v] = Bb
            sums[A] -= deg[v]
            sums[Bb] += deg[v]
            cnts[A] -= 1
            cnts[Bb] += 1
        else:
            _, v1, v2 = best
            assign[v1], assign[v2] = Bb, A
            sums[A] += deg[v2] - deg[v1]
            sums[Bb] += deg[v1] - deg[v2]
    assert np.all(sums == P) and np.all(cnts <= VCAP), "block packing failed"

    blocks = []
    for b in range(NBLK):
        vlist = sorted(v for v in assign if assign[v] == b)
        pe = np.concatenate([edges_of[v] for v in vlist])
        assert pe.size == P
        blocks.append((pe, np.array(vlist)))
    return blocks


def _build_nc(need_clamp):
    nc = bacc.Bacc("TRN2", target_bir_lowering=False, debug=False,
                   num_devices=NCORES)
    W = NBLK * CHUNK  # free-dim width of one chunk's rhs/out region
    NH = 2 * NSLOT  # half-slots: 2 blocks each, 64 llr partitions
    LW = NH * CHUNK  # free-dim width of one chunk's llr region
    WXE = BOFF + 2 * NBLK * P  # wem: [u bits, lg bits, (w0|m0), (w1|m1), ...]
    HEAD = BOFF + 8 * 2 * P  # u/lg + first 8 blocks (2 PSUM groups)
    rhsp = nc.dram_tensor("rhsp", [P, NCHUNK * W], F8, kind="ExternalInput").ap()
    llrp = nc.dram_tensor("llrp", [64, NCHUNK * LW], F16, kind="ExternalInput").ap()
    wem = nc.dram_tensor("wem", [P, WXE], F16, kind="ExternalInput").ap()
    wvm = nc.dram_tensor("wvm", [64, 2 * NH * P], F16, kind="ExternalInput").ap()
    outp = nc.dram_tensor("outp", [P, NCHUNK * W], F16, kind="ExternalOutput").ap()

    with TileContext(nc) as tc, ExitStack() as ctx:
        cpool = ctx.enter_context(tc.tile_pool(name="const", bufs=1))
        xpool = ctx.enter_context(tc.tile_pool(name="x8", bufs=2))
        lpool = ctx.enter_context(tc.tile_pool(name="llr", bufs=2))
        opool = ctx.enter_context(tc.tile_pool(name="out", bufs=4))
        pspool = ctx.enter_context(tc.tile_pool(name="ps", bufs=2, space="PSUM"))

        # weights: head (u/lg + 2 groups) + var weights on the sync ring;
        # the edge-weight tail arrives on the gpsimd ring
        wt = cpool.tile([P, WXE], F16)
        nc.sync.dma_start(wt[:, 0:HEAD], wem[:, 0:HEAD])
        wvt = cpool.tile([64, 2 * NH * P], F16)
        nc.sync.dma_start(wvt[:], wvm[:])
        nc.gpsimd.dma_start(wt[:, HEAD:WXE], wem[:, HEAD:WXE])

        # PE warmup: zero matmuls during the input ramp so the HAM clock
        # gate releases (1.2 -> 2.4 GHz) before the real matmuls start
        zl = cpool.tile([P, P], F16)
        nc.gpsimd.memset(zl[:], 0.0)
        zr = cpool.tile([P, CHUNK], F16)
        nc.gpsimd.memset(zr[:], 0.0)
        wps = pspool.tile([P, 4 * CHUNK], F32, tag="ps")
        for _ in range(NWARM):
            nc.tensor.matmul(wps[:, 0:CHUNK], zl[:], zr[:], start=True, stop=True)

        # z = (u < sigmoid(dropout_logits)) in fp32.  u/logits arrive as raw
        # fp32 bit patterns in the fp16 weight tensor; a DVE copy feeds ACT
        # a clean f32 tile (ACT cannot take bitcast APs)
        zt = cpool.tile([P, NBLK], F32)
        nc.vector.tensor_copy(zt[:], wt[:, 2 * NBLK : 4 * NBLK].bitcast(F32))
        nc.scalar.activation(zt[:], zt[:], AF.Sigmoid)
        nc.vector.tensor_tensor(
            zt[:], wt[:, 0 : 2 * NBLK].bitcast(F32), zt[:], ALU.is_lt)

        # skip weights: llr_weights * smask, one fused DVE op for all blocks
        nc.vector.tensor_tensor(
            wvt[:, 0 : NH * P], wvt[:, 0 : NH * P],
            wvt[:, NH * P : 2 * NH * P], ALU.mult)
        # edge weights: (odd_weights * z[row]) * mask, one fused op per block
        for g in range(NBLK):
            sl = wt[:, BOFF + 2 * g * P : BOFF + (2 * g + 1) * P]
            msl = wt[:, BOFF + (2 * g + 1) * P : BOFF + (2 * g + 2) * P]
            nc.vector.scalar_tensor_tensor(
                sl, sl, zt[:, g : g + 1], msl, ALU.mult, ALU.mult)

        for nb in range(NCHUNK):
            xt = xpool.tile([P, W], F8)
            nc.sync.dma_start(xt[:], rhsp[:, nb * W : (nb + 1) * W])
            lt = lpool.tile([64, LW], F16)
            nc.sync.dma_start(lt[:], llrp[:, nb * LW : (nb + 1) * LW])
            for q in range(NBLK // 4):
                ps = pspool.tile([P, 4 * CHUNK], F32)
                for i in range(4):
                    g = 4 * q + i
                    psl = ps[:, i * CHUNK : (i + 1) * CHUNK]
                    nc.tensor.matmul(
                        psl,
                        wt[:, BOFF + 2 * g * P : BOFF + (2 * g + 1) * P],
                        xt[:, g * CHUNK : (g + 1) * CHUNK],
                        start=True, stop=False,
                    )
                    h = 2 * q + i // 2
                    o = VCAP * (i % 2)
                    nc.tensor.matmul(
                        psl,
                        wvt[o : o + VCAP, h * P : (h + 1) * P],
                        lt[o : o + VCAP, h * CHUNK : (h + 1) * CHUNK],
                        start=False, stop=True,
                    )
                ot = opool.tile([P, 4 * CHUNK], F16)
                if need_clamp:
                    nc.vector.tensor_scalar(
                        ot[:], ps[:], 10.0, -10.0, ALU.min, ALU.max)
                    nc.scalar.activation(ot[:], ot[:], AF.Tanh, scale=0.5)
                else:
                    # clip(v, +-10) proven identity for these inputs (see
                    # bound in _prep); tanh straight from PSUM
                    nc.scalar.activation(ot[:], ps[:], AF.Tanh, scale=0.5)
                c0 = nb * W + q * 4 * CHUNK
                # stores via gpsimd/SWDGE keep the per-DMA issue cost off
                # the ACT engine; the last chunk stores on the sync HWDGE
                # ring for a short tail
                if nb == NCHUNK - 1:
                    nc.sync.dma_start(outp[:, c0 : c0 + 4 * CHUNK], ot[:])
                else:
                    nc.gpsimd.dma_start(outp[:, c0 : c0 + 4 * CHUNK], ot[:])
    nc.compile()
    return nc


def _prep(x, llr, u, odd_weights, llr_weights, dropout_logits,
          w_odd2even_mask, w_skipconn2even_mask):
    """Host-side data movement: block packing, block gathers, shards, casts."""
    ow = np.asarray(odd_weights, np.float32)
    msk = np.asarray(w_odd2even_mask, np.float32)
    lw = np.asarray(llr_weights, np.float32)
    smask = np.asarray(w_skipconn2even_mask, np.float32)
    u = np.asarray(u, np.float32)
    lg = np.asarray(dropout_logits, np.float32)

    blocks = _plan_blocks(smask)

    wE = np.zeros((P, NBLK * P), np.float32)
    mE = np.zeros((P, NBLK * P), np.float32)
    NH = 2 * NSLOT
    wV = np.zeros((64, NH * P), np.float32)
    mV = np.zeros((64, NH * P), np.float32)
    ucomb = np.zeros((P, NBLK), np.float32)
    lgcomb = np.zeros((P, NBLK), np.float32)
    rows_src = np.zeros(NBLK * P, np.int64)  # rhs row (g,p) -> edge id
    lsrc = np.full(NH * 64, -1, np.int64)  # llr row (h,p) -> var id
    for g, (pe, vs) in enumerate(blocks):
        s, j = g // 4, g % 4
        h = 2 * s + j // 2
        o = VCAP * (j % 2)
        wE[:, g * P : (g + 1) * P] = ow[np.ix_(pe, pe)]
        mE[:, g * P : (g + 1) * P] = msk[np.ix_(pe, pe)]
        nv = len(vs)
        wV[o : o + nv, h * P : (h + 1) * P] = lw[np.ix_(vs, pe)]
        mV[o : o + nv, h * P : (h + 1) * P] = smask[np.ix_(vs, pe)]
        ucomb[:, g] = u[pe]
        lgcomb[:, g] = lg[pe]
        rows_src[g * P : (g + 1) * P] = pe
        lsrc[h * 64 + o : h * 64 + o + nv] = vs

    x = np.asarray(x, np.float32)
    llr = np.asarray(llr, np.float32)
    x8 = x.astype(ml_dtypes.float8_e3m4)
    llr16 = llr.astype(np.float16)

    # Rigorous bound on |msg + skip|: if it cannot reach the +-10 clip,
    # the clip is the identity and the device clamp stage is elided.
    xmax = float(np.abs(x8.astype(np.float32)).max())
    lmax = float(np.abs(llr16.astype(np.float32)).max())
    awE = np.abs(wE.astype(np.float16).astype(np.float32)
                 * mE).reshape(P, NBLK, P).sum(axis=0)  # [NBLK, P] per column
    awV = np.abs(wV.astype(np.float16).astype(np.float32) * mV)
    bound = 0.0
    for g in range(NBLK):
        s, j = g // 4, g % 4
        h = 2 * s + j // 2
        o = VCAP * (j % 2)
        v_sum = awV[o : o + VCAP, h * P : (h + 1) * P].sum(axis=0)
        bound = max(bound, float((awE[g] * xmax + v_sum * lmax).max()))
    need_clamp = bound >= 9.5

    # wem: raw fp32 bit patterns of u and logits (viewed as 2 fp16 each;
    # device bitcasts them back to fp32), then per block the fp16 weight
    # block followed by its mask block
    w16 = wE.astype(np.float16)
    m16 = mE.astype(np.float16)
    parts = [ucomb.view(np.float16), lgcomb.view(np.float16)]
    for g in range(NBLK):
        parts.append(w16[:, g * P : (g + 1) * P])
        parts.append(m16[:, g * P : (g + 1) * P])
    wem = np.ascontiguousarray(np.concatenate(parts, axis=1))
    assert wem.shape == (P, BOFF + 2 * NBLK * P)
    wvm = np.ascontiguousarray(np.concatenate(
        [wV.astype(np.float16), mV.astype(np.float16)], axis=1))

    in_maps = []
    for c in range(NCORES):
        sl = slice(c * BSH, (c + 1) * BSH)
        xs = x8[sl].T[rows_src]  # [NBLK*128, BSH] fp8
        rhsp = np.ascontiguousarray(
            xs.reshape(NBLK, P, NCHUNK, CHUNK).transpose(1, 2, 0, 3)
        ).reshape(P, NCHUNK * NBLK * CHUNK)
        ls = np.zeros((NH * 64, BSH), np.float16)
        valid = lsrc >= 0
        ls[valid] = llr16[sl].T[lsrc[valid]]
        llrp = np.ascontiguousarray(
            ls.reshape(NH, 64, NCHUNK, CHUNK).transpose(1, 2, 0, 3)
        ).reshape(64, NCHUNK * NH * CHUNK)
        in_maps.append({
            "rhsp": rhsp,
            "llrp": llrp,
            "wem": wem,
            "wvm": wvm,
        })
    return blocks, in_maps, need_clamp


def _run(inputs: dict, trace: bool = False, **kwargs):
    blocks, in_maps, need_clamp = _prep(**inputs)
    nc = _build_nc(need_clamp)
    res = run_bass_kernel_spmd(nc, in_maps, list(range(NCORES)), trace=trace, **kwargs)

    # decode: outp [128, NCHUNK, NBLK, CHUNK] -> rows (g, p) -> edge column
    dest = np.concatenate([pe for pe, _ in blocks])
    out = np.empty((B, E), np.float32)
    for c in range(NCORES):
        sl = slice(c * BSH, (c + 1) * BSH)
        arr = (res.results[c]["outp"]
               .astype(np.float32)
               .reshape(P, NCHUNK, NBLK, CHUNK)
               .transpose(2, 0, 1, 3)
               .reshape(NBLK * P, BSH))
        out[sl][:, dest] = arr.T
    return out, res


def kernel(**inputs) -> np.ndarray:
    out, _ = _run(inputs, trace=False)
    return out
